# revision 1
# baseline (speedup 1.0000x reference)
"""DCGRU cell on 8 Trainium2 NeuronCores.

Sharding: data-parallel over batch (B=32 -> 4 per core), adjacency + MLP
weights replicated. No collectives; host gathers per-core outputs.

Per-core layouts (all f32):
  node-major (nm): [16 tiles][128 nodes, 768] cols = b*192+f   (diffusion lhsT)
  feat-major (fm): [6 tiles][128 bf-rows, 2048 nodes]          (hop outputs, MLP rhs)
Hop matmul: out_fm[bf, i] = sum_j x_nm[j, bf] * W[i, j]
  = matmul(lhsT=x_nm[jt][:, c*128:+128], rhs=WT[jt][:, i-block]) accumulated
  over jt in PSUM, so W is streamed host-pretransposed (WT[j, i] = W[i, j]).
MLP: gate logits acc[b][o, n] += WxI[k][bf, o].T @ fm[k][bf-slice, n] with
  batch-interleaved host-packed weights WxI (rows = b*192+f), accumulated
  across hops in DRAM via accum_op=add DMAs straight from PSUM.
Chain re-entry: fm -> nm via PE transposes (hops 1,2 of each direction only).
"""

import sys
import numpy as np
import ml_dtypes

for _p in ("/opt/trn_rl_repo",):
    if _p not in sys.path:
        sys.path.insert(0, _p)

from concourse import bacc, tile, mybir  # noqa: E402
from concourse.alu_op_type import AluOpType as ALU  # noqa: E402
from concourse.bass_utils import run_bass_kernel_spmd  # noqa: E402

F32 = mybir.dt.float32
F32R = mybir.dt.float32r
BF16 = mybir.dt.bfloat16
MM_BF16 = True          # matmul datapath dtype: True -> bf16, False -> f32r
MMDT = BF16 if MM_BF16 else F32R
AF = mybir.ActivationFunctionType

C = 4          # batches per core
FI = 192       # per-batch feature width (x 64 + h 128)
BF = C * FI    # 768
DH = 128
NCORES = 8
NHOPS = 3


def build_nc(nt=16):
    """Build + compile the per-core Bass kernel. nt = node tiles (N = nt*128)."""
    N = nt * 128
    nbk = N // 512

    nc = bacc.Bacc("TRN2", target_bir_lowering=False, debug=False,
                   num_devices=NCORES)

    def din(name, shape, dt=F32):
        return nc.dram_tensor(name, shape, dt, kind="ExternalInput").ap()

    XH = din("xh_nm", [nt, 128, BF], MMDT)
    XHFM = din("xh_fm", [6, 128, N], MMDT)
    WFT = din("wfT", [nt, 128, N], MMDT)
    WBT = din("wbT", [nt, 128, N], MMDT)
    WRI = din("wrI", [7, 3, 64, 128], MMDT)
    WZI = din("wzI", [7, 3, 64, 128], MMDT)
    WNI = din("wnI", [7, 3, 64, 128], MMDT)
    XFM = din("x_fm", [C, 64, N], MMDT)
    HFM = din("h_fm", [C, 128, N])
    BR = din("br_c", [128, 1])
    BZ = din("bz_c", [128, 1])
    BN = din("bn_c", [128, 1])
    IDT = din("ident", [128, 128], MMDT)
    OUT = nc.dram_tensor("out_fm", [C, 128, N], F32, kind="ExternalOutput").ap()

    ACCR = nc.dram_tensor("acc_r", [C, 128, N], F32).ap()
    ACCZ = nc.dram_tensor("acc_z", [C, 128, N], F32).ap()
    ACCN = nc.dram_tensor("acc_n", [C, 128, N], F32).ap()
    XRH = nc.dram_tensor("xrh_nm_d", [nt, 128, BF], MMDT).ap()

    with tile.TileContext(nc) as tc:
        with (
            tc.tile_pool(name="nm", bufs=32) as nm_pool,
            tc.tile_pool(name="fm", bufs=12) as fm_pool,
            tc.tile_pool(name="gate", bufs=4) as gate_pool,
            tc.tile_pool(name="wt", bufs=6) as wt_pool,
            tc.tile_pool(name="wxi", bufs=18) as wxi_pool,
            tc.tile_pool(name="aux", bufs=12) as aux_pool,
            tc.tile_pool(name="stg", bufs=4) as stg_pool,
            tc.tile_pool(name="const", bufs=1) as const_pool,
            tc.tile_pool(name="ps", bufs=6, space="PSUM") as ps_pool,
            tc.tile_pool(name="psx", bufs=2, space="PSUM") as psx_pool,
        ):
            ident = const_pool.tile([128, 128], MMDT, tag="ident")
            nc.sync.dma_start(ident[:], IDT[:])
            brt = const_pool.tile([128, 1], F32, tag="brt")
            nc.sync.dma_start(brt[:], BR[:])
            bzt = const_pool.tile([128, 1], F32, tag="bzt")
            nc.sync.dma_start(bzt[:], BZ[:])
            bnt = const_pool.tile([128, 1], F32, tag="bnt")
            nc.sync.dma_start(bnt[:], BN[:])

            def load_nm(SRC):
                ts = []
                for jt in range(nt):
                    t = nm_pool.tile([128, BF], MMDT, name="nmt", tag="nm")
                    nc.sync.dma_start(t[:], SRC[jt])
                    ts.append(t)
                return ts

            def hop(src, WT):
                """One diffusion hop; returns fm tiles (6 x [128, N])."""
                fms = [fm_pool.tile([128, N], MMDT, name="fmt", tag="fm") for _ in range(6)]
                for ibk in range(nbk):
                    pss = [ps_pool.tile([128, 512], F32, name="pst", tag="ps")
                           for _ in range(6)]
                    for jt in range(nt):
                        wt = wt_pool.tile([128, 512], MMDT, name="wtt", tag="wt")
                        nc.sync.dma_start(
                            wt[:], WT[jt][:, 512 * ibk:512 * (ibk + 1)])
                        for c in range(6):
                            nc.tensor.matmul(
                                pss[c][:],
                                src[jt][:, 128 * c:128 * (c + 1)],
                                wt[:],
                                start=(jt == 0), stop=(jt == nt - 1))
                    for c in range(6):
                        nc.vector.tensor_copy(
                            fms[c][:, 512 * ibk:512 * (ibk + 1)], pss[c][:])
                return fms

            def aux_of(fms):
                """Base-0 copies of rows [64:128) of each fm tile (so every
                MLP contraction segment sits at partition 0 -> one PSUM
                accumulation group, no mixed tile_position)."""
                auxs = []
                for t in range(6):
                    a = aux_pool.tile([64, N], MMDT, name="auxt", tag="aux")
                    nc.gpsimd.dma_start(a[:], fms[t][64:128, :])
                    auxs.append(a)
                return auxs

            def mlp_feed(fms, auxs, kidx, gates, first):
                """gates: list of (WXI dram, ACC dram). Accumulate logits."""
                for WXI, ACCD in gates:
                    wx = []
                    for s in range(3):
                        w = wxi_pool.tile([64, 128], MMDT, name="wxit", tag="wxi")
                        nc.gpsimd.dma_start(w[:], WXI[kidx][s])
                        wx.append(w)
                    for b in range(C):
                        for nb in range(nbk):
                            nbs = slice(512 * nb, 512 * (nb + 1))
                            ps = psx_pool.tile([128, 512], F32, name="psxt", tag="psx")
                            for s in range(3):
                                t, off = divmod(b * FI + 64 * s, 128)
                                rhs = (fms[t][0:64, nbs] if off == 0
                                       else auxs[t][0:64, nbs])
                                nc.tensor.matmul(ps[:], wx[s][:], rhs,
                                                 start=(s == 0), stop=(s == 2))
                            stg = stg_pool.tile([128, 512], F32, name="stgt", tag="stg")
                            nc.vector.tensor_copy(stg[:], ps[:])
                            nc.gpsimd.dma_start(
                                ACCD[b][:, nbs], stg[:],
                                accum_op=(ALU.bypass if first else ALU.add))

            def retranspose(fms):
                """fm tiles -> fresh nm tiles via PE transposes."""
                nms = [nm_pool.tile([128, BF], MMDT, name="nmt", tag="nm")
                       for _ in range(nt)]
                for it in range(nt):
                    ps = psx_pool.tile([128, BF], MMDT, name="psxt", tag="psx")
                    for c in range(6):
                        nc.tensor.transpose(
                            ps[:, 128 * c:128 * (c + 1)],
                            fms[c][:, 128 * it:128 * (it + 1)],
                            ident[:])
                    nc.vector.tensor_copy(nms[it][:], ps[:])
                return nms

            def diffusion(x_nm_loader, x_fm_tiles, gates, xnm_first=None):
                """Full 2-direction diffusion + MLP accumulation.
                MLP feeds are deferred one hop so they never gate the next
                hop's matmul stream (fm pool holds 2 chunks)."""
                mlp_feed(x_fm_tiles, aux_of(x_fm_tiles), 0, gates,
                         first=True)
                pending = None
                cur = xnm_first if xnm_first is not None else x_nm_loader()
                for wdir, WT in ((0, WFT), (1, WBT)):
                    if wdir == 1:
                        cur = x_nm_loader()
                    for k in range(1, NHOPS + 1):
                        fm = hop(cur, WT)
                        aux = aux_of(fm)
                        cur = retranspose(fm) if k < NHOPS else None
                        if pending is not None:
                            mlp_feed(*pending)
                        pending = (fm, aux, wdir * NHOPS + k, gates, False)
                mlp_feed(*pending)

            # ---------------- diffusion 1 (r, z gates) ----------------
            fm0 = []
            for t in range(6):
                f = fm_pool.tile([128, N], MMDT, name="fmt", tag="fm")
                nc.scalar.dma_start(f[:], XHFM[t])
                fm0.append(f)
            diffusion(lambda: load_nm(XH), fm0, [(WRI, ACCR), (WZI, ACCZ)])

            # ------------- gates r, z; assemble xrh (nm + fm) -------------
            xrh_nm = [nm_pool.tile([128, BF], MMDT, name="nmt", tag="nm")
                      for _ in range(nt)]
            xrh_fm = [fm_pool.tile([128, N], MMDT, name="fmt", tag="fm") for _ in range(6)]
            for b in range(C):
                accr = gate_pool.tile([128, N], F32, name="gatet", tag="gate")
                nc.scalar.dma_start(accr[:], ACCR[b])
                r = gate_pool.tile([128, N], F32, name="gatet", tag="gate")
                nc.scalar.activation(r[:], accr[:], AF.Sigmoid, bias=brt[:])
                h = gate_pool.tile([128, N], F32, name="gatet", tag="gate")
                nc.scalar.dma_start(h[:], HFM[b])
                rh = fm_pool.tile([128, N], MMDT, name="fmt", tag="fm")
                nc.vector.tensor_mul(rh[:], r[:], h[:])
                # rh columns of xrh_nm (PE transpose 128-blocks)
                for g in range(nt // 4):
                    ps = psx_pool.tile([128, 512], MMDT, name="psxt", tag="psx")
                    for q in range(4):
                        it = 4 * g + q
                        nc.tensor.transpose(
                            ps[:, 128 * q:128 * (q + 1)],
                            rh[:, 128 * it:128 * (it + 1)], ident[:])
                    for q in range(4):
                        nc.vector.tensor_copy(
                            xrh_nm[4 * g + q][:, b * FI + 64:(b + 1) * FI],
                            ps[:, 128 * q:128 * (q + 1)])
                # fm rows of xrh: x piece then two rh 64-row pieces
                t, off = divmod(b * FI, 128)
                nc.scalar.dma_start(xrh_fm[t][off:off + 64, :], XFM[b])
                for s2 in range(2):
                    t, off = divmod(b * FI + 64 + 64 * s2, 128)
                    nc.scalar.dma_start(xrh_fm[t][off:off + 64, :],
                                        rh[64 * s2:64 * (s2 + 1), :])
            # x columns of xrh_nm straight from the xh param
            for jt in range(nt):
                for b in range(C):
                    nc.scalar.dma_start(xrh_nm[jt][:, b * FI:b * FI + 64],
                                        XH[jt][:, b * FI:b * FI + 64])
            # spill xrh_nm for the backward-chain reload
            for jt in range(nt):
                nc.sync.dma_start(XRH[jt], xrh_nm[jt][:])

            # ---------------- diffusion 2 (n gate) ----------------
            diffusion(lambda: load_nm(XRH), xrh_fm, [(WNI, ACCN)],
                      xnm_first=xrh_nm)

            # ---------------- final gate ----------------
            for b in range(C):
                accn = gate_pool.tile([128, N], F32, name="gatet", tag="gate")
                nc.scalar.dma_start(accn[:], ACCN[b])
                n_t = gate_pool.tile([128, N], F32, name="gatet", tag="gate")
                nc.scalar.activation(n_t[:], accn[:], AF.Tanh, bias=bnt[:])
                h = gate_pool.tile([128, N], F32, name="gatet", tag="gate")
                nc.scalar.dma_start(h[:], HFM[b])
                accz = gate_pool.tile([128, N], F32, name="gatet", tag="gate")
                nc.scalar.dma_start(accz[:], ACCZ[b])
                z = gate_pool.tile([128, N], F32, name="gatet", tag="gate")
                nc.scalar.activation(z[:], accz[:], AF.Sigmoid, bias=bzt[:])
                d = gate_pool.tile([128, N], F32, name="gatet", tag="gate")
                nc.vector.tensor_sub(d[:], n_t[:], h[:])
                zd2 = gate_pool.tile([128, N], F32, name="gatet", tag="gate")
                nc.vector.tensor_mul(zd2[:], z[:], d[:])
                o = gate_pool.tile([128, N], F32, name="gatet", tag="gate")
                nc.vector.tensor_add(o[:], zd2[:], h[:])
                nc.scalar.dma_start(OUT[b], o[:])

    nc.compile()
    return nc


def _pack_interleaved(W):
    """[128, 7*192] torch-Linear weight -> [7, 3, 64, 128] transposed 64-row
    contraction segments: out[k, s, f, o] = W[o, k*192 + 64*s + f]."""
    out = np.zeros((7, 3, 64, 128), np.float32)
    for k in range(7):
        for s in range(3):
            out[k, s] = W[:, k * FI + 64 * s:k * FI + 64 * (s + 1)].T
    return np.ascontiguousarray(out)


_NC_CACHE = {}


def _get_nc(nt):
    if nt not in _NC_CACHE:
        _NC_CACHE[nt] = build_nc(nt)
    return _NC_CACHE[nt]


def make_in_maps(x, h_prev, W_fwd, W_bwd, Wr, br, Wz, bz, Wn, bn):
    mdt = np.dtype(ml_dtypes.bfloat16) if MM_BF16 else np.float32
    x = np.asarray(x, np.float32)
    h_prev = np.asarray(h_prev, np.float32)
    B, N, Din = x.shape
    nt = N // 128
    WfT = np.ascontiguousarray(np.asarray(W_fwd, np.float32).T).reshape(nt, 128, N)
    WbT = np.ascontiguousarray(np.asarray(W_bwd, np.float32).T).reshape(nt, 128, N)
    wrI = _pack_interleaved(np.asarray(Wr, np.float32))
    wzI = _pack_interleaved(np.asarray(Wz, np.float32))
    wnI = _pack_interleaved(np.asarray(Wn, np.float32))
    ident = np.ascontiguousarray(np.eye(128, dtype=np.float32))
    WfT_d = WfT.astype(mdt)
    WbT_d = WbT.astype(mdt)
    wrI_d = wrI.astype(mdt)
    wzI_d = wzI.astype(mdt)
    wnI_d = wnI.astype(mdt)
    ident_d = ident.astype(mdt)
    brc = np.ascontiguousarray(np.asarray(br, np.float32).reshape(128, 1))
    bzc = np.ascontiguousarray(np.asarray(bz, np.float32).reshape(128, 1))
    bnc = np.ascontiguousarray(np.asarray(bn, np.float32).reshape(128, 1))
    ncores = B // C
    in_maps = []
    for cix in range(ncores):
        xs = x[C * cix:C * (cix + 1)]
        hs = h_prev[C * cix:C * (cix + 1)]
        xh = np.concatenate([xs, hs], axis=-1)            # [C, N, 192]
        flat = np.ascontiguousarray(xh.transpose(1, 0, 2).reshape(N, BF))
        xh_nm = np.ascontiguousarray(flat).reshape(nt, 128, BF)
        xh_fm = np.ascontiguousarray(flat.T).reshape(6, 128, N)
        x_fm = np.ascontiguousarray(xs.transpose(0, 2, 1))
        h_fm = np.ascontiguousarray(hs.transpose(0, 2, 1))
        in_maps.append(dict(
            xh_nm=xh_nm.astype(mdt), xh_fm=xh_fm.astype(mdt),
            wfT=WfT_d, wbT=WbT_d, wrI=wrI_d, wzI=wzI_d, wnI=wnI_d,
            x_fm=x_fm.astype(mdt), h_fm=h_fm,
            br_c=brc, bz_c=bzc, bn_c=bnc, ident=ident_d))
    return in_maps, nt, ncores


def kernel(x, h_prev, W_fwd, W_bwd, Wr, br, Wz, bz, Wn, bn, _trace=False):
    in_maps, nt, ncores = make_in_maps(
        x, h_prev, W_fwd, W_bwd, Wr, br, Wz, bz, Wn, bn)
    nc = _get_nc(nt)
    res = run_bass_kernel_spmd(nc, in_maps, list(range(ncores)), trace=_trace)
    outs = [np.ascontiguousarray(res.results[c]["out_fm"].transpose(0, 2, 1))
            for c in range(ncores)]
    full = np.concatenate(outs, axis=0).astype(np.float32)
    if _trace:
        return full, res
    return full



# revision 3
# speedup vs baseline: 1.1814x; 1.1814x over previous
"""DCGRU cell on 8 Trainium2 NeuronCores.

Sharding: data-parallel over batch (B=32 -> 4 per core), adjacency + MLP
weights replicated. No collectives; host gathers per-core outputs.

Per-core layouts (all bf16 matmul datapath, f32 accum):
  x node-major:  [16 tiles][128 nodes, 256] cols = b*64+fx    (diffusion lhsT)
  h node-major:  [16 tiles][128 nodes, 512] cols = b*128+fh
  x feat-major:  [2 tiles][128 rows=b*64+fx, 2048 nodes]      (hop outputs, MLP rhs)
  h feat-major:  [4 tiles][128 rows=b*128+fh, 2048 nodes]
Hop matmul: out_fm[bf, i] = sum_j x_nm[j, bf] * W[i, j]
  = matmul(lhsT=nm[jt][:, c*128:+128], rhs=WT[jt][:, i-block]) accumulated
  over jt in PSUM; W streamed host-pretransposed (WT[j, i] = W[i, j]).
x/h segregation pays off twice:
  - diffusion 2 hops only r*h (the x columns of [x | r*h] are unchanged
    from diffusion 1); diff-1 x-hop outputs are spilled to DRAM and
    reloaded for diffusion-2 MLP feeds.
  - MLP per (b, n-block) is one K=128 matmul (h segment, full tile) plus
    one K=64 matmul (x segment); the K=64s of a batch pair sit at row
    offsets 0/64 and run concurrently on disjoint PE row groups.
MLP: gate logits acc[b][o, n] accumulated across hops in DRAM via
  accum_op=add DMAs straight from PSUM staging.
Chain re-entry: fm -> nm via PE transposes (hops 1,2 of each direction).
"""

import sys
import numpy as np
import ml_dtypes

for _p in ("/opt/trn_rl_repo",):
    if _p not in sys.path:
        sys.path.insert(0, _p)

from concourse import bacc, tile, mybir  # noqa: E402
from concourse.alu_op_type import AluOpType as ALU  # noqa: E402
from concourse.bass_utils import run_bass_kernel_spmd  # noqa: E402

F32 = mybir.dt.float32
BF16 = mybir.dt.bfloat16
MMDT = BF16
AF = mybir.ActivationFunctionType

C = 4          # batches per core
DX = 64        # x features per batch
DH = 128       # h features per batch
BFX = C * DX   # 256
BFH = C * DH   # 512
NCORES = 8
NHOPS = 3


def build_nc(nt=16):
    """Build + compile the per-core Bass kernel. nt = node tiles (N = nt*128)."""
    N = nt * 128
    nbk = N // 512
    NXT = BFX // 128   # 2 x fm tiles
    NHT = BFH // 128   # 4 h fm tiles

    nc = bacc.Bacc("TRN2", target_bir_lowering=False, debug=False,
                   num_devices=NCORES)

    def din(name, shape, dt=F32):
        return nc.dram_tensor(name, shape, dt, kind="ExternalInput").ap()

    XNM = din("x_nm", [nt, 128, BFX], MMDT)
    HNM = din("h_nm", [nt, 128, BFH], MMDT)
    XFM0 = din("x_fm0", [NXT, 128, N], MMDT)
    HFM0 = din("h_fm0", [NHT, 128, N], MMDT)
    HFMF = din("h_fmf", [C, 128, N])
    WFT = din("wfT", [nt, 128, N], MMDT)
    WBT = din("wbT", [nt, 128, N], MMDT)
    WRX = din("wrx", [7, 128, 128], MMDT)
    WRH = din("wrh", [7, 128, 128], MMDT)
    WZX = din("wzx", [7, 128, 128], MMDT)
    WZH = din("wzh", [7, 128, 128], MMDT)
    WNX = din("wnx", [7, 128, 128], MMDT)
    WNH = din("wnh", [7, 128, 128], MMDT)
    BR = din("br_c", [128, 1])
    BZ = din("bz_c", [128, 1])
    BN = din("bn_c", [128, 1])
    IDT = din("ident", [128, 128], MMDT)
    OUT = nc.dram_tensor("out_fm", [C, 128, N], F32, kind="ExternalOutput").ap()

    ACCR = nc.dram_tensor("acc_r", [C, 128, N], F32).ap()
    ACCZ = nc.dram_tensor("acc_z", [C, 128, N], F32).ap()
    ACCN = nc.dram_tensor("acc_n", [C, 128, N], F32).ap()
    XFMS = nc.dram_tensor("xfm_sp", [2 * NHOPS, NXT, 128, N], MMDT).ap()
    RHNM = nc.dram_tensor("rh_nm_d", [nt, 128, BFH], MMDT).ap()

    with tile.TileContext(nc) as tc:
        with (
            tc.tile_pool(name="xnm", bufs=32) as xnm_pool,
            tc.tile_pool(name="hnm", bufs=32) as hnm_pool,
            tc.tile_pool(name="xfm", bufs=7) as xfm_pool,
            tc.tile_pool(name="hfm", bufs=13) as hfm_pool,
            tc.tile_pool(name="gate", bufs=4) as gate_pool,
            tc.tile_pool(name="wt", bufs=6) as wt_pool,
            tc.tile_pool(name="wxi", bufs=8) as wxi_pool,
            tc.tile_pool(name="stg", bufs=4) as stg_pool,
            tc.tile_pool(name="const", bufs=1) as const_pool,
            tc.tile_pool(name="ps", bufs=6, space="PSUM") as ps_pool,
            tc.tile_pool(name="psx", bufs=2, space="PSUM") as psx_pool,
        ):
            ident = const_pool.tile([128, 128], MMDT, tag="ident")
            nc.sync.dma_start(ident[:], IDT[:])
            brt = const_pool.tile([128, 1], F32, tag="brt")
            nc.sync.dma_start(brt[:], BR[:])
            bzt = const_pool.tile([128, 1], F32, tag="bzt")
            nc.sync.dma_start(bzt[:], BZ[:])
            bnt = const_pool.tile([128, 1], F32, tag="bnt")
            nc.sync.dma_start(bnt[:], BN[:])

            def load_xnm():
                ts = []
                for jt in range(nt):
                    t = xnm_pool.tile([128, BFX], MMDT, name="xnmt", tag="xnm")
                    nc.sync.dma_start(t[:], XNM[jt])
                    ts.append(t)
                return ts

            def load_hnm(SRC):
                ts = []
                for jt in range(nt):
                    t = hnm_pool.tile([128, BFH], MMDT, name="hnmt", tag="hnm")
                    nc.sync.dma_start(t[:], SRC[jt])
                    ts.append(t)
                return ts

            def hop(src_x, src_h, WT):
                """One diffusion hop; x part optional.
                Returns (xfm 2x[128,N] | None, hfm 4x[128,N])."""
                fx = ([xfm_pool.tile([128, N], MMDT, name="xfmt", tag="xfm")
                       for _ in range(NXT)] if src_x is not None else None)
                fh = [hfm_pool.tile([128, N], MMDT, name="hfmt", tag="hfm")
                      for _ in range(NHT)]
                ncs = (NXT if src_x is not None else 0) + NHT
                for ibk in range(nbk):
                    pss = [ps_pool.tile([128, 512], F32, name="pst", tag="ps")
                           for _ in range(ncs)]
                    for jt in range(nt):
                        wt = wt_pool.tile([128, 512], MMDT, name="wtt", tag="wt")
                        nc.sync.dma_start(
                            wt[:], WT[jt][:, 512 * ibk:512 * (ibk + 1)])
                        pi = 0
                        if src_x is not None:
                            for c in range(NXT):
                                nc.tensor.matmul(
                                    pss[pi][:],
                                    src_x[jt][:, 128 * c:128 * (c + 1)],
                                    wt[:],
                                    start=(jt == 0), stop=(jt == nt - 1))
                                pi += 1
                        for c in range(NHT):
                            nc.tensor.matmul(
                                pss[pi][:],
                                src_h[jt][:, 128 * c:128 * (c + 1)],
                                wt[:],
                                start=(jt == 0), stop=(jt == nt - 1))
                            pi += 1
                    pi = 0
                    if src_x is not None:
                        for c in range(NXT):
                            nc.vector.tensor_copy(
                                fx[c][:, 512 * ibk:512 * (ibk + 1)], pss[pi][:])
                            pi += 1
                    for c in range(NHT):
                        nc.vector.tensor_copy(
                            fh[c][:, 512 * ibk:512 * (ibk + 1)], pss[pi][:])
                        pi += 1
                return fx, fh

            def mlp_feed(fx, fh, kidx, gates, first):
                """gates: list of (WX, WH, ACC dram). Accumulate logits.
                fx: 2 x [128,N] (x segment, rows b*64+fx), fh: 4 x [128,N].
                Per (b, nb): K=128 matmul on fh[b] + K=64 on half of fx[b//2];
                the two K=64s of a batch pair run on disjoint PE row groups."""
                for WX, WH, ACCD in gates:
                    wx = wxi_pool.tile([128, 128], MMDT, name="wxt", tag="wxi")
                    nc.gpsimd.dma_start(wx[:], WX[kidx])
                    wh = wxi_pool.tile([128, 128], MMDT, name="wht", tag="wxi")
                    nc.gpsimd.dma_start(wh[:], WH[kidx])
                    for nb in range(nbk):
                        nbs = slice(512 * nb, 512 * (nb + 1))
                        for bp in range(2):
                            b0, b1 = 2 * bp, 2 * bp + 1
                            p0 = psx_pool.tile([128, 512], F32, name="psxt", tag="psx")
                            p1 = psx_pool.tile([128, 512], F32, name="psxt", tag="psx")
                            nc.tensor.matmul(p0[:], wh[:], fh[b0][:, nbs],
                                             start=True, stop=False)
                            nc.tensor.matmul(p1[:], wh[:], fh[b1][:, nbs],
                                             start=True, stop=False)
                            nc.tensor.matmul(p0[:], wx[0:64, :],
                                             fx[bp][0:64, nbs],
                                             start=False, stop=True)
                            nc.tensor.matmul(p1[:], wx[64:128, :],
                                             fx[bp][64:128, nbs],
                                             start=False, stop=True)
                            for b, ps in ((b0, p0), (b1, p1)):
                                stg = stg_pool.tile([128, 512], F32,
                                                    name="stgt", tag="stg")
                                nc.vector.tensor_copy(stg[:], ps[:])
                                nc.gpsimd.dma_start(
                                    ACCD[b][:, nbs], stg[:],
                                    accum_op=(ALU.bypass if first else ALU.add))

            def retranspose(fx, fh):
                """fm tiles -> fresh nm tiles via PE transposes."""
                nxs = ([xnm_pool.tile([128, BFX], MMDT, name="xnmt", tag="xnm")
                        for _ in range(nt)] if fx is not None else None)
                nhs = [hnm_pool.tile([128, BFH], MMDT, name="hnmt", tag="hnm")
                       for _ in range(nt)]
                for it in range(nt):
                    its = slice(128 * it, 128 * (it + 1))
                    if fx is not None:
                        px = psx_pool.tile([128, BFX], MMDT, name="psxt", tag="psx")
                        for c in range(NXT):
                            nc.tensor.transpose(
                                px[:, 128 * c:128 * (c + 1)], fx[c][:, its],
                                ident[:])
                        nc.vector.tensor_copy(nxs[it][:], px[:])
                    ph = psx_pool.tile([128, BFH], MMDT, name="psxt", tag="psx")
                    for c in range(NHT):
                        nc.tensor.transpose(
                            ph[:, 128 * c:128 * (c + 1)], fh[c][:, its],
                            ident[:])
                    nc.vector.tensor_copy(nhs[it][:], ph[:])
                return nxs, nhs

            def load_xfm_spill(kidx):
                ts = []
                for c in range(NXT):
                    t = xfm_pool.tile([128, N], MMDT, name="xfmt", tag="xfm")
                    nc.scalar.dma_start(t[:], XFMS[kidx - 1][c])
                    ts.append(t)
                return ts

            # ---------------- diffusion 1 (r, z gates) ----------------
            gates1 = [(WRX, WRH, ACCR), (WZX, WZH, ACCZ)]
            xfm0 = []
            for c in range(NXT):
                t = xfm_pool.tile([128, N], MMDT, name="xfmt", tag="xfm")
                nc.scalar.dma_start(t[:], XFM0[c])
                xfm0.append(t)
            hfm0 = []
            for c in range(NHT):
                t = hfm_pool.tile([128, N], MMDT, name="hfmt", tag="hfm")
                nc.scalar.dma_start(t[:], HFM0[c])
                hfm0.append(t)

            pending = (xfm0, hfm0, 0, gates1, True)
            cur_x, cur_h = load_xnm(), load_hnm(HNM)
            for wdir, WT in ((0, WFT), (1, WBT)):
                if wdir == 1:
                    cur_x, cur_h = load_xnm(), load_hnm(HNM)
                for k in range(1, NHOPS + 1):
                    fx, fh = hop(cur_x, cur_h, WT)
                    kidx = wdir * NHOPS + k
                    for c in range(NXT):
                        nc.sync.dma_start(XFMS[kidx - 1][c], fx[c][:])
                    if k < NHOPS:
                        cur_x, cur_h = retranspose(fx, fh)
                    else:
                        cur_x = cur_h = None
                    if pending is not None:
                        mlp_feed(*pending)
                    pending = (fx, fh, kidx, gates1, False)
            mlp_feed(*pending)

            # ------------- gates r, z; assemble rh (nm + fm) -------------
            rh_fm = []
            rh_nm = [hnm_pool.tile([128, BFH], MMDT, name="hnmt", tag="hnm")
                     for _ in range(nt)]
            for b in range(C):
                accr = gate_pool.tile([128, N], F32, name="gatet", tag="gate")
                nc.scalar.dma_start(accr[:], ACCR[b])
                r = gate_pool.tile([128, N], F32, name="gatet", tag="gate")
                nc.scalar.activation(r[:], accr[:], AF.Sigmoid, bias=brt[:])
                h = gate_pool.tile([128, N], F32, name="gatet", tag="gate")
                nc.scalar.dma_start(h[:], HFMF[b])
                rh = hfm_pool.tile([128, N], MMDT, name="hfmt", tag="hfm")
                nc.vector.tensor_mul(rh[:], r[:], h[:])
                rh_fm.append(rh)
                # rh columns of rh_nm (PE transpose 128-blocks)
                for g in range(nt // 4):
                    ps = psx_pool.tile([128, 512], MMDT, name="psxt", tag="psx")
                    for q in range(4):
                        it = 4 * g + q
                        nc.tensor.transpose(
                            ps[:, 128 * q:128 * (q + 1)],
                            rh[:, 128 * it:128 * (it + 1)], ident[:])
                    for q in range(4):
                        nc.vector.tensor_copy(
                            rh_nm[4 * g + q][:, b * DH:(b + 1) * DH],
                            ps[:, 128 * q:128 * (q + 1)])
            # spill rh_nm for the backward-chain reload
            for jt in range(nt):
                nc.sync.dma_start(RHNM[jt], rh_nm[jt][:])

            # ---------------- diffusion 2 (n gate) ----------------
            gates2 = [(WNX, WNH, ACCN)]
            xfm0b = []
            for c in range(NXT):
                t = xfm_pool.tile([128, N], MMDT, name="xfmt", tag="xfm")
                nc.scalar.dma_start(t[:], XFM0[c])
                xfm0b.append(t)
            pending = (xfm0b, rh_fm, 0, gates2, True)
            cur_h = rh_nm
            for wdir, WT in ((0, WFT), (1, WBT)):
                if wdir == 1:
                    cur_h = load_hnm(RHNM)
                for k in range(1, NHOPS + 1):
                    _, fh = hop(None, cur_h, WT)
                    kidx = wdir * NHOPS + k
                    if k < NHOPS:
                        _, cur_h = retranspose(None, fh)
                    else:
                        cur_h = None
                    if pending is not None:
                        mlp_feed(*pending)
                    pending = (load_xfm_spill(kidx), fh, kidx, gates2, False)
            mlp_feed(*pending)

            # ---------------- final gate ----------------
            for b in range(C):
                accn = gate_pool.tile([128, N], F32, name="gatet", tag="gate")
                nc.scalar.dma_start(accn[:], ACCN[b])
                n_t = gate_pool.tile([128, N], F32, name="gatet", tag="gate")
                nc.scalar.activation(n_t[:], accn[:], AF.Tanh, bias=bnt[:])
                h = gate_pool.tile([128, N], F32, name="gatet", tag="gate")
                nc.scalar.dma_start(h[:], HFMF[b])
                accz = gate_pool.tile([128, N], F32, name="gatet", tag="gate")
                nc.scalar.dma_start(accz[:], ACCZ[b])
                z = gate_pool.tile([128, N], F32, name="gatet", tag="gate")
                nc.scalar.activation(z[:], accz[:], AF.Sigmoid, bias=bzt[:])
                d = gate_pool.tile([128, N], F32, name="gatet", tag="gate")
                nc.vector.tensor_sub(d[:], n_t[:], h[:])
                zd2 = gate_pool.tile([128, N], F32, name="gatet", tag="gate")
                nc.vector.tensor_mul(zd2[:], z[:], d[:])
                o = gate_pool.tile([128, N], F32, name="gatet", tag="gate")
                nc.vector.tensor_add(o[:], zd2[:], h[:])
                nc.scalar.dma_start(OUT[b], o[:])

    nc.compile()
    return nc


def _pack_gate(W):
    """[128, 7*192] torch-Linear weight -> (WX [7,128,128], WH [7,128,128]):
    WX[k] = [Wx_k.T ; Wx_k.T] (dup 64-row x segment on both row groups),
    WH[k] = Wh_k.T (128-row h segment)."""
    wx = np.zeros((7, 128, 128), np.float32)
    wh = np.zeros((7, 128, 128), np.float32)
    for k in range(7):
        xs = W[:, k * 192:k * 192 + DX].T          # [64,128]
        wx[k][0:64] = xs
        wx[k][64:128] = xs
        wh[k] = W[:, k * 192 + DX:(k + 1) * 192].T  # [128,128]
    return np.ascontiguousarray(wx), np.ascontiguousarray(wh)


_NC_CACHE = {}


def _get_nc(nt):
    if nt not in _NC_CACHE:
        _NC_CACHE[nt] = build_nc(nt)
    return _NC_CACHE[nt]


def make_in_maps(x, h_prev, W_fwd, W_bwd, Wr, br, Wz, bz, Wn, bn):
    mdt = np.dtype(ml_dtypes.bfloat16)
    x = np.asarray(x, np.float32)
    h_prev = np.asarray(h_prev, np.float32)
    B, N, Din = x.shape
    nt = N // 128
    WfT = np.ascontiguousarray(np.asarray(W_fwd, np.float32).T).reshape(nt, 128, N)
    WbT = np.ascontiguousarray(np.asarray(W_bwd, np.float32).T).reshape(nt, 128, N)
    wrx, wrh = _pack_gate(np.asarray(Wr, np.float32))
    wzx, wzh = _pack_gate(np.asarray(Wz, np.float32))
    wnx, wnh = _pack_gate(np.asarray(Wn, np.float32))
    ident = np.ascontiguousarray(np.eye(128, dtype=np.float32))
    WfT_d = WfT.astype(mdt)
    WbT_d = WbT.astype(mdt)
    packs = dict(
        wrx=wrx.astype(mdt), wrh=wrh.astype(mdt),
        wzx=wzx.astype(mdt), wzh=wzh.astype(mdt),
        wnx=wnx.astype(mdt), wnh=wnh.astype(mdt))
    brc = np.ascontiguousarray(np.asarray(br, np.float32).reshape(128, 1))
    bzc = np.ascontiguousarray(np.asarray(bz, np.float32).reshape(128, 1))
    bnc = np.ascontiguousarray(np.asarray(bn, np.float32).reshape(128, 1))
    ncores = B // C
    in_maps = []
    for cix in range(ncores):
        xs = x[C * cix:C * (cix + 1)]          # [C,N,64]
        hs = h_prev[C * cix:C * (cix + 1)]     # [C,N,128]
        # node-major: [N, C*F] col = b*F+f
        x_nm = np.ascontiguousarray(
            xs.transpose(1, 0, 2).reshape(N, BFX)).reshape(nt, 128, BFX)
        h_nm = np.ascontiguousarray(
            hs.transpose(1, 0, 2).reshape(N, BFH)).reshape(nt, 128, BFH)
        # feat-major: rows b*F+f
        x_fm0 = np.ascontiguousarray(
            xs.transpose(0, 2, 1).reshape(BFX, N)).reshape(2, 128, N)
        h_fmf = np.ascontiguousarray(hs.transpose(0, 2, 1))  # [C,128,N]
        h_fm0 = h_fmf.reshape(4, 128, N)
        in_maps.append(dict(
            x_nm=x_nm.astype(mdt), h_nm=h_nm.astype(mdt),
            x_fm0=x_fm0.astype(mdt), h_fm0=h_fm0.astype(mdt),
            h_fmf=h_fmf,
            wfT=WfT_d, wbT=WbT_d, **packs,
            br_c=brc, bz_c=bzc, bn_c=bnc, ident=ident.astype(mdt)))
    return in_maps, nt, ncores


def kernel(x, h_prev, W_fwd, W_bwd, Wr, br, Wz, bz, Wn, bn, _trace=False):
    in_maps, nt, ncores = make_in_maps(
        x, h_prev, W_fwd, W_bwd, Wr, br, Wz, bz, Wn, bn)
    nc = _get_nc(nt)
    res = run_bass_kernel_spmd(nc, in_maps, list(range(ncores)), trace=_trace)
    outs = [np.ascontiguousarray(res.results[c]["out_fm"].transpose(0, 2, 1))
            for c in range(ncores)]
    full = np.concatenate(outs, axis=0).astype(np.float32)
    if _trace:
        return full, res
    return full
